# revision 5
# baseline (speedup 1.0000x reference)
"""Trainium2 Bass kernel for quantized-MoE Bottleneck (nn_Bottleneck_37503654429269).

v4 design (one core = 4 samples, SPMD over 8 cores, data-parallel on batch):
- Host-side: expert routing, weight quantization (exact bf16 integers), and
  x-quantization (exact integer levels in bf16) per sample.
- Device: conv1 -> bn1-affine (ACT) -> exact round via +-2^24 on DVE ->
  clamp on POOL -> padded conv2 (9-tap shifted matmuls) -> bn2 likewise ->
  conv3 -> fused drain (+GN sums via ACT
  accum_out) + sum-of-squares via DVE scalar_tensor_tensor accum -> tiny GN
  math -> P/Q outer products on PE (gnb folded in) -> affine_then_add with
  the fp16 residual -> POOL relu -> fp16 store.
- Engine split: PE matmuls; ACT bn-affines + conv3 drains + small drains;
  DVE clamps + sumsq + GN math + final affine; POOL final relu.
"""

import numpy as np

BITS = (2, 4, 8)
EPS = 1e-5
B, C_IN, H, W = 32, 1024, 14, 14
WIDTH, OUTC = 256, 1024
PIX = H * W  # 196
NCORES = 8
RB24 = 12582912.0  # 1.5*2^23: classic fp32 round-to-int magic (both signs)

_NC_CACHE = {}


# ----------------------------------------------------------------------------
# Device program
# ----------------------------------------------------------------------------

def _build_nc(group_sizes):
    from contextlib import ExitStack
    import concourse.bacc as bacc
    import concourse.mybir as mybir
    import concourse.tile as tile

    F32 = mybir.dt.float32
    BF16 = mybir.dt.bfloat16
    FP16 = mybir.dt.float16
    ALU = mybir.AluOpType
    ACT = mybir.ActivationFunctionType

    NG = len(group_sizes)
    NS = sum(group_sizes)
    assert NS == 4
    slot0 = [sum(group_sizes[:g]) for g in range(NG)]
    # chunks of <=2 samples (local index within group)
    chunks = []  # (g, c0_local, nchunk)
    for g in range(NG):
        for c0 in range(0, group_sizes[g], 2):
            chunks.append((g, c0, min(2, group_sizes[g] - c0)))

    nc = bacc.Bacc("TRN2", target_bir_lowering=False, debug=False,
                   num_devices=NCORES)

    # ---- dram tensors
    xq_d = nc.dram_tensor("xq", [128, 8, 4 * PIX], BF16, kind="ExternalInput")
    x_d = nc.dram_tensor("x", [128, 8, 4 * PIX], FP16, kind="ExternalInput")
    w1_d = nc.dram_tensor("w1", [128, NG, 8, 256], BF16, kind="ExternalInput")
    w2_d = nc.dram_tensor("w2", [128, NG, 9, 2, 256], BF16,
                          kind="ExternalInput")
    w3_d = nc.dram_tensor("w3", [128, NG, 2, 1024], BF16,
                          kind="ExternalInput")
    # per-partition consts, 10 per group:
    # A1[2] B1[2] A2[2] B2[2] C3E XB
    cc_d = nc.dram_tensor("cc", [128, 10 * NG], F32, kind="ExternalInput")
    # rows: gng[1024] gnb[1024] ones[8]
    gr_d = nc.dram_tensor("gr", [1, 2056], F32, kind="ExternalInput")
    out_d = nc.dram_tensor("out", [128, 8, 4 * PIX], FP16,
                           kind="ExternalOutput")

    with tile.TileContext(nc) as tc, ExitStack() as ctx:
        res = ctx.enter_context(tc.tile_pool(name="res", bufs=1))
        rot = ctx.enter_context(tc.tile_pool(name="rot", bufs=4))
        mm1 = ctx.enter_context(tc.tile_pool(name="mm1", bufs=2, space="PSUM"))
        mm2 = ctx.enter_context(tc.tile_pool(name="mm2", bufs=2, space="PSUM"))
        mm3 = ctx.enter_context(tc.tile_pool(name="mm3", bufs=2, space="PSUM"))
        pqp = ctx.enter_context(tc.tile_pool(name="pqp", bufs=1, space="PSUM"))
        rdp = ctx.enter_context(tc.tile_pool(name="rdp", bufs=1, space="PSUM"))

        # ---- loads (order = DMA priority)
        CC = res.tile([128, 10 * NG], F32, name="CC", tag="CC")
        nc.sync.dma_start(out=CC, in_=cc_d.ap())
        GR = res.tile([1, 2056], F32, name="GR", tag="GR")
        nc.sync.dma_start(out=GR, in_=gr_d.ap())
        GNG = GR[:, 0:1024]
        GNB = GR[:, 1024:2048]

        W1 = res.tile([128, NG, 8, 256], BF16, name="W1", tag="W1")
        nc.sync.dma_start(out=W1, in_=w1_d.ap())
        XQ = [res.tile([128, 2, 4 * PIX], BF16, name=f"XQ{h}", tag=f"XQ{h}")
              for h in range(4)]
        for h in range(4):
            nc.sync.dma_start(out=XQ[h], in_=xq_d.ap()[:, 2 * h:2 * h + 2, :])
        W2 = res.tile([128, NG, 9, 2, 256], BF16, name="W2", tag="W2")
        nc.sync.dma_start(out=W2, in_=w2_d.ap())
        W3 = res.tile([128, NG, 2, 1024], BF16, name="W3", tag="W3")
        nc.sync.dma_start(out=W3, in_=w3_d.ap())
        X = res.tile([128, 8, 4 * PIX], FP16, name="X", tag="X")
        nc.sync.dma_start(out=X, in_=x_d.ap())

        def XQv(kt):
            return XQ[kt // 2][:, kt % 2, :]

        ONESC = res.tile([128, 1], F32, name="ONESC", tag="ONESC")
        nc.vector.memset(ONESC, 1.0)

        def A1(g, ko):
            return CC[:, 10 * g + ko:10 * g + ko + 1]

        def B1(g, ko):
            return CC[:, 10 * g + 2 + ko:10 * g + 3 + ko]

        def A2(g, ko):
            return CC[:, 10 * g + 4 + ko:10 * g + 5 + ko]

        def B2(g, ko):
            return CC[:, 10 * g + 6 + ko:10 * g + 7 + ko]

        def C3E(g):
            return CC[:, 10 * g + 8:10 * g + 9]

        def XB(g):
            return CC[:, 10 * g + 9:10 * g + 10]

        # padded conv2 inputs, per (ko, g): [128, ns, 16, 18]
        HP = [[res.tile([128, group_sizes[g], 16, 18], BF16,
                        name=f"HP{ko}_{g}", tag=f"HP{ko}_{g}")
               for g in range(NG)] for ko in range(2)]
        for ko in range(2):
            for g in range(NG):
                nc.vector.memset(HP[ko][g], 0.0)

        # conv2 outputs (quantized), per (ko, g): [128, ns*196]
        Q2 = [[res.tile([128, group_sizes[g] * PIX], BF16,
                        name=f"Q2{ko}_{g}", tag=f"Q2{ko}_{g}")
               for g in range(NG)] for ko in range(2)]

        # conv3 outputs + stats + final out, per chunk
        H3 = [res.tile([128, 8, nchunk * PIX], FP16, name=f"H3_{ci}",
                       tag=f"H3_{ci}")
              for ci, (g, c0, nchunk) in enumerate(chunks)]
        STAT = [res.tile([128, 2, 8, nchunk], F32, name=f"STAT_{ci}",
                         tag=f"STAT_{ci}")
                for ci, (g, c0, nchunk) in enumerate(chunks)]
        OUT = [res.tile([128, 8, nchunk * PIX], FP16, name=f"OUT_{ci}",
                        tag=f"OUT_{ci}")
               for ci, (g, c0, nchunk) in enumerate(chunks)]

        # ---------------- conv1 + bn1 + qact ----------------
        for g, c0, nchunk in chunks:
            cols = slice((slot0[g] + c0) * PIX, (slot0[g] + c0 + nchunk) * PIX)
            for ko in range(2):
                ps = mm1.tile([128, nchunk * PIX], F32, name="c1ps", tag="c1")
                for kt in range(8):
                    nc.tensor.matmul(
                        ps,
                        W1[:, g, kt, ko * 128:(ko + 1) * 128],
                        XQv(kt)[:, cols],
                        start=(kt == 0), stop=(kt == 7))
                # u = bn1 affine (f32), exact round on DVE, clamp on POOL
                u = rot.tile([128, nchunk * PIX], F32, name="u1", tag="u1")
                nc.scalar.activation(out=u, in_=ps, func=ACT.Identity,
                                     bias=B1(g, ko), scale=A1(g, ko))
                r = rot.tile([128, nchunk * PIX], BF16, name="r1", tag="r1")
                nc.vector.tensor_scalar(out=r, in0=u, scalar1=RB24,
                                        scalar2=RB24, op0=ALU.add,
                                        op1=ALU.subtract)
                nc.gpsimd.tensor_scalar(
                    out=HP[ko][g][:, c0:c0 + nchunk, 1:15, 2:16],
                    in0=r.rearrange("p (s y x) -> p s y x", s=nchunk, y=14),
                    scalar1=0.0, scalar2=XB(g),
                    op0=ALU.max, op1=ALU.min)

        # ---------------- conv2 + bn2 + qact ----------------
        for g, c0, nchunk in chunks:
            for ko in range(2):
                ps = mm2.tile([128, nchunk, 14, 14], F32, name="c2ps",
                              tag="c2")
                first = True
                for ti, (dy, dx) in enumerate(
                        (dy, dx) for dy in range(3) for dx in range(3)):
                    for kt in range(2):
                        nc.tensor.matmul(
                            ps,
                            W2[:, g, ti, kt, ko * 128:(ko + 1) * 128],
                            HP[kt][g][:, c0:c0 + nchunk,
                                      dy:dy + 14, dx + 1:dx + 15],
                            start=first, stop=(ti == 8 and kt == 1))
                        first = False
                u = rot.tile([128, nchunk * PIX], F32, name="u2", tag="u2")
                nc.scalar.activation(out=u,
                                     in_=ps.rearrange("p s y x -> p (s y x)"),
                                     func=ACT.Identity,
                                     bias=B2(g, ko), scale=A2(g, ko))
                r = rot.tile([128, nchunk * PIX], BF16, name="r2", tag="r2")
                nc.vector.tensor_scalar(out=r, in0=u, scalar1=RB24,
                                        scalar2=RB24, op0=ALU.add,
                                        op1=ALU.subtract)
                nc.gpsimd.tensor_scalar(
                    out=Q2[ko][g][:, c0 * PIX:(c0 + nchunk) * PIX],
                    in0=r, scalar1=0.0, scalar2=XB(g),
                    op0=ALU.max, op1=ALU.min)

        # ---------------- conv3 + drains + moment accums ----------------
        for ci, (g, c0, nchunk) in enumerate(chunks):
            for mo in range(8):
                ps = mm3.tile([128, nchunk * PIX], F32, name="c3ps", tag="c3")
                for kt in range(2):
                    nc.tensor.matmul(
                        ps,
                        W3[:, g, kt, mo * 128:(mo + 1) * 128],
                        Q2[kt][g][:, c0 * PIX:(c0 + nchunk) * PIX],
                        start=(kt == 0), stop=(kt == 1))
                for si in range(nchunk):
                    h3s = H3[ci][:, mo, si * PIX:(si + 1) * PIX]
                    nc.scalar.activation(
                        out=h3s, in_=ps[:, si * PIX:(si + 1) * PIX],
                        func=ACT.Copy, bias=0.0, scale=C3E(g),
                        accum_out=STAT[ci][:, 0, mo, si:si + 1])
                    sq = rot.tile([128, PIX], FP16, name="sq", tag="sq")
                    nc.vector.scalar_tensor_tensor(
                        out=sq, in0=h3s, scalar=1.0, in1=h3s,
                        op0=ALU.mult, op1=ALU.mult,
                        accum_out=STAT[ci][:, 1, mo, si:si + 1])

        # ---------------- per-chunk GN tail ----------------
        NINV = 1.0 / (2 * 128 * PIX)
        for ci, (g, c0, nchunk) in enumerate(chunks):
            nstat = 2 * 8 * nchunk
            red = rdp.tile([1, nstat], F32, name="red", tag="red")
            nc.tensor.matmul(red, ONESC,
                             STAT[ci].rearrange("p a b c -> p (a b c)"),
                             start=True, stop=True)
            # Tg layout [1, kind(2), gi(4), mopar(2), s]
            Tg = rot.tile([1, 2, 4, 2, nchunk], F32, name="Tg", tag="Tg")
            nc.scalar.activation(out=Tg.rearrange("p a b c d -> p (a b c d)"),
                                 in_=red, func=ACT.Copy,
                                 bias=0.0, scale=1.0)
            TB = rot.tile([1, 2, 4, nchunk], F32, name="TB", tag="TB")
            nc.vector.tensor_tensor(out=TB, in0=Tg[:, :, :, 0, :],
                                    in1=Tg[:, :, :, 1, :], op=ALU.add)
            nc.vector.tensor_scalar(
                out=TB.rearrange("p a b c -> p (a b c)"),
                in0=TB.rearrange("p a b c -> p (a b c)"),
                scalar1=NINV, scalar2=None, op0=ALU.mult)
            ms = TB[:, 0, :, :]
            es = TB[:, 1, :, :]
            VAR = rot.tile([1, 4, nchunk], F32, name="VAR", tag="VAR")
            nc.vector.tensor_tensor(out=VAR, in0=ms, in1=ms, op=ALU.mult)
            nc.vector.tensor_tensor(out=VAR, in0=es, in1=VAR, op=ALU.subtract)
            SD = rot.tile([1, 4 * nchunk], F32, name="SD", tag="SD")
            nc.scalar.activation(out=SD,
                                 in_=VAR.rearrange("p a b -> p (a b)"),
                                 func=ACT.Sqrt, bias=GR[:, 2054:2055],
                                 scale=1.0)
            AB = rot.tile([1, 2, 4, nchunk], F32, name="AB", tag="AB")
            nc.vector.reciprocal(
                out=AB[:, 0, :, :].rearrange("p a b -> p (a b)"), in_=SD)
            nc.vector.scalar_tensor_tensor(
                out=AB[:, 1, :, :], in0=ms, scalar=-1.0, in1=AB[:, 0, :, :],
                op0=ALU.mult, op1=ALU.mult)

            # P/Q columns: P = gng*A ; Q = gng*B + gnb
            pq = pqp.tile([128, 8, 2, nchunk], F32, name="pq", tag="pq")
            nmm = 0
            for mo in range(8):
                nc.tensor.matmul(
                    pq[:, mo, :, :],
                    GNG[:, mo * 128:(mo + 1) * 128],
                    AB[:, :, mo // 2, :],
                    start=(nmm == 0), stop=False, skip_group_check=True)
                nmm += 1
                nc.tensor.matmul(
                    pq[:, mo, 1, :],
                    GNB[:, mo * 128:(mo + 1) * 128],
                    GR[:, 2048:2048 + nchunk],
                    start=False, stop=(mo == 7), skip_group_check=True)
            PQD = rot.tile([128, 8, 2, nchunk], F32, name="PQD", tag="PQD")
            nc.scalar.activation(
                out=PQD.rearrange("p a b c -> p (a b c)"),
                in_=pq.rearrange("p a b c -> p (a b c)"),
                func=ACT.Copy, bias=0.0, scale=1.0)

            for mo in range(8):
                for si in range(nchunk):
                    slot = slot0[g] + c0 + si
                    ov = OUT[ci][:, mo, si * PIX:(si + 1) * PIX]
                    nc.vector.affine_then_add(
                        out=ov,
                        in0=H3[ci][:, mo, si * PIX:(si + 1) * PIX],
                        in1=X[:, mo, slot * PIX:(slot + 1) * PIX],
                        scale=PQD[:, mo, 0, si:si + 1],
                        bias=PQD[:, mo, 1, si:si + 1])
                    nc.gpsimd.tensor_relu(out=ov, in_=ov)
            nc.sync.dma_start(
                out=out_d.ap()[:, :, (slot0[g] + c0) * PIX:
                               (slot0[g] + c0 + nchunk) * PIX],
                in_=OUT[ci])

    nc.compile()
    return nc


# ----------------------------------------------------------------------------
# Host side
# ----------------------------------------------------------------------------

def _quant_w(w, lv):
    n = max(lv // 2 - 1, 1)
    s = np.float32(np.abs(w).max()) + np.float32(1e-12)
    k = np.round((w.astype(np.float32) / s) * np.float32(n)).astype(np.float32)
    return k, np.float32(s) / np.float32(n)


def _assign_groups(mask):
    mask = np.asarray(mask).astype(np.int64)
    ids = {e: [int(i) for i in np.nonzero(mask == e)[0]] for e in range(3)}
    counts = [len(ids[e]) for e in range(3)]
    if all(c % 2 == 0 for c in counts):
        group_sizes = (2, 2)
        chunks2 = []
        for e in range(3):
            for j in range(0, counts[e], 2):
                chunks2.append((e, ids[e][j:j + 2]))
        assert len(chunks2) == 16
        core_samples = []
        core_experts = []
        for c in range(8):
            (ea, sa), (eb, sb) = chunks2[2 * c], chunks2[2 * c + 1]
            core_samples.append(sa + sb)
            core_experts.append([ea, eb])
        return group_sizes, core_samples, core_experts

    base = [c % 3 for c in counts]
    need = (8 - sum(base)) // 3
    t = [0, 0, 0]
    for e in range(3):
        cap = (counts[e] - base[e]) // 3
        take = min(cap, need)
        t[e] = take
        need -= take
        if need == 0:
            break
    assert need == 0
    b = [base[e] + 3 * t[e] for e in range(3)]
    a = [(counts[e] - b[e]) // 3 for e in range(3)]
    assert sum(a) == 8 and sum(b) == 8
    trip = []
    single = []
    for e in range(3):
        pos = 0
        for _ in range(a[e]):
            trip.append((e, ids[e][pos:pos + 3]))
            pos += 3
        for _ in range(b[e]):
            single.append((e, [ids[e][pos]]))
            pos += 1
        assert pos == counts[e]
    core_samples = []
    core_experts = []
    for c in range(8):
        ea, sa = trip[c]
        eb, sb = single[c]
        core_samples.append(sa + sb)
        core_experts.append([ea, eb])
    return (3, 1), core_samples, core_experts


def kernel(x, mask, w1, w2, w3, bn1_g, bn1_b, bn1_m, bn1_v,
           bn2_g, bn2_b, bn2_m, bn2_v, gn_g, gn_b):
    import ml_dtypes
    from concourse.bass_utils import run_bass_kernel_spmd

    bf16 = ml_dtypes.bfloat16
    f16 = np.float16
    f32 = np.float32
    x = np.asarray(x, f32)
    mask = np.asarray(mask)
    w1 = np.asarray(w1, f32)
    w2 = np.asarray(w2, f32)
    w3 = np.asarray(w3, f32)
    bn1 = [np.asarray(v, f32) for v in (bn1_g, bn1_b, bn1_m, bn1_v)]
    bn2 = [np.asarray(v, f32) for v in (bn2_g, bn2_b, bn2_m, bn2_v)]
    gn_g = np.asarray(gn_g, f32)
    gn_b = np.asarray(gn_b, f32)

    group_sizes, core_samples, core_experts = _assign_groups(mask)
    NG = len(group_sizes)
    slot0 = [sum(group_sizes[:g]) for g in range(NG)]

    lv_of = [2 ** b for b in BITS]
    K1, K2, K3 = {}, {}, {}
    CW = {}
    for e in set(int(v) for v in np.asarray(mask)):
        lv = lv_of[e]
        k1, c1 = _quant_w(w1, lv)
        k2, c2 = _quant_w(w2, lv)
        k3, c3 = _quant_w(w3, lv)
        K1[e] = k1.reshape(256, 1024)
        K2[e] = k2.reshape(256, 256, 3, 3)
        K3[e] = k3.reshape(1024, 256)
        CW[e] = (c1, c2, c3)

    inv1 = bn1[0] / np.sqrt(bn1[3] + f32(EPS))
    bb1 = bn1[1] - bn1[2] * inv1
    inv2 = bn2[0] / np.sqrt(bn2[3] + f32(EPS))
    bb2 = bn2[1] - bn2[2] * inv2

    def pack_w(e):
        k1t = K1[e].T.reshape(8, 128, 256).transpose(1, 0, 2)
        k2t = K2[e].transpose(2, 3, 1, 0).reshape(9, 2, 128, 256)
        k2t = k2t.transpose(2, 0, 1, 3)
        k3t = K3[e].T.reshape(2, 128, 1024).transpose(1, 0, 2)
        return (np.ascontiguousarray(k1t).astype(bf16),
                np.ascontiguousarray(k2t).astype(bf16),
                np.ascontiguousarray(k3t).astype(bf16))

    packed = {e: pack_w(e) for e in K1}

    in_maps = []
    for c in range(8):
        sids = core_samples[c]
        experts = core_experts[c]
        glv = [lv_of[experts[g]] for g in range(NG)]

        # residual x: [128, mo, 4*196] fp16
        xc = x[sids].reshape(4, 8, 128, PIX).transpose(2, 1, 0, 3) \
                    .reshape(128, 8, 4 * PIX)
        # quantized x per sample (exact integer levels)
        xqs = np.empty((4, C_IN, PIX), f32)
        for g in range(NG):
            lv = glv[g]
            for si in range(group_sizes[g]):
                t = slot0[g] + si
                xs = x[sids[t]].reshape(C_IN, PIX)
                xqs[t] = np.clip(np.round(xs * f32(lv - 1)), 0.0,
                                 f32(lv - 1))
        xqc = xqs.reshape(4, 8, 128, PIX).transpose(2, 1, 0, 3) \
                 .reshape(128, 8, 4 * PIX)

        w1c = np.stack([packed[experts[g]][0] for g in range(NG)], axis=1)
        w2c = np.stack([packed[experts[g]][1] for g in range(NG)], axis=1)
        w3c = np.stack([packed[experts[g]][2] for g in range(NG)], axis=1)

        cc = np.zeros((128, 10 * NG), f32)
        for g in range(NG):
            e = experts[g]
            lv = glv[g]
            c1, c2, c3 = CW[e]
            cc[:, 10 * g + 0:10 * g + 2] = (inv1 * c1).reshape(2, 128).T
            cc[:, 10 * g + 2:10 * g + 4] = \
                (bb1 * f32(lv - 1)).reshape(2, 128).T
            cc[:, 10 * g + 4:10 * g + 6] = (inv2 * c2).reshape(2, 128).T
            cc[:, 10 * g + 6:10 * g + 8] = \
                (bb2 * f32(lv - 1)).reshape(2, 128).T
            cc[:, 10 * g + 8] = c3 / f32(lv - 1)
            cc[:, 10 * g + 9] = f32(lv - 1)

        gr = np.zeros((1, 2056), f32)
        gr[0, 0:1024] = gn_g
        gr[0, 1024:2048] = gn_b
        gr[0, 2048:2054] = 1.0
        gr[0, 2054] = f32(EPS)

        in_maps.append({
            "xq": xqc.astype(bf16), "x": xc.astype(f16),
            "w1": w1c, "w2": w2c, "w3": w3c, "cc": cc, "gr": gr,
        })

    key = group_sizes
    if key not in _NC_CACHE:
        _NC_CACHE[key] = _build_nc(group_sizes)
    nc = _NC_CACHE[key]

    res = run_bass_kernel_spmd(nc, in_maps, core_ids=list(range(NCORES)))

    out = np.zeros((B, OUTC, H, W), f32)
    for c in range(8):
        oc = res.results[c]["out"].astype(f32)  # [128, 8, 4*PIX]
        oc = oc.reshape(128, 8, 4, PIX).transpose(2, 1, 0, 3) \
               .reshape(4, OUTC, H, W)
        for t, sid in enumerate(core_samples[c]):
            out[sid] = oc[t]
    return out
